# revision 5
# baseline (speedup 1.0000x reference)
"""Trainium2 Bass kernel: batched 64-digit base-10 addition (nn_Adder).

The reference RNN scan is just carry-propagating decimal addition:
    s_e = a_e + b_e; v_e = s_e + c_e; c_{e+1} = [v_e >= 10];
    digit_e = v_e mod 10   (digits stored MSB-first, carries run LSB->MSB)

Mapping onto one NeuronCore (pure data parallel across 8 cores, batch
524288 -> 65536 rows/core). The kernel is HBM-bound (3 x 16.78 MB per
core); all compute runs on the DVE, comfortably under the DMA time, so
the pipeline streams at memory speed:

  * G=32 rows are packed per SBUF partition along the free dim, with a
    zero separator column before each 64-digit group. At a separator the
    scan state is the previous group's carry (0 or 1) < 10, so the carry
    into the next group's LSB is 0 -> ONE tensor_tensor_scan instruction
    carries 128*G rows.
  * s = a + b is a single DVE tensor_tensor add writing straight into
    the LSB-first separator layout (the MSB<->LSB reversal is folded
    into the output access pattern).
  * DVE runs the carry scan
        v_t = [10 <= v_{t-1}] + s_t      (op0=is_le, op1=add)
    with bf16 output (values <= 19, exact), then digit extraction in
    bf16 fast modes: tensor_scalar m = -10*[v >= 10] (4x mode) and an
    in-place tensor_tensor digit = m + v (2x mode). The DVE ISA has no
    mod op (codegen ISA check rejects it), so digit = v - 10*carry.
  * ACT (ScalarEngine, own SBUF ports) upcasts the bf16 digits to the
    f32 output tile, folding the LSB->MSB reversal into its input
    access pattern, and issues the output DMAs from its queue.
  * PE / ACT / GpSimd are deliberately idle: the TensorEngine identity-
    matmul variant of s=a+b made PE the critical path (155us busy), and
    any GpSimd op grabs the DVE shared SBUF port pair and degrades
    concurrent DVE ops ~3x.

All values are small integers, exact in f32 -> bit-exact output.
"""

import sys

sys.path.insert(0, "/opt/trn_rl_repo")

import numpy as np

BATCH = 524288
SEQ = 64
N_CORES = 8
B_LOC = BATCH // N_CORES

P = 128
GS = SEQ + 1        # group stride in s/w tiles (64 digits + 1 separator)
# per-tile digit-rows-per-partition schedule: small tiles at both ends
# shorten pipeline fill and the end-of-kernel drain
G_LIST = [8, 8, 16] + [32] * 14 + [16, 8, 8]
G_MAX = max(G_LIST)
IO_BUFS = 4
WK_BUFS = 3
N_SPP = 3           # ping-pong buffers for the separator-layout s tile

_nc_cache = {}


def _build_adder():
    from contextlib import ExitStack

    import concourse.bacc as bacc
    import concourse.bass as bass
    import concourse.mybir as mybir
    import concourse.tile as tile

    F32 = mybir.dt.float32
    BF16 = mybir.dt.bfloat16
    ALU = mybir.AluOpType
    ACTF = mybir.ActivationFunctionType

    assert P * sum(G_LIST) == B_LOC
    FD = G_MAX * SEQ    # max data cols in a/b/d tiles
    FS = G_MAX * GS + 1 # max cols in s/w tiles

    nc = bacc.Bacc("TRN2", target_bir_lowering=False, debug=False)
    a_ext = nc.declare_dram_parameter("a", [B_LOC, SEQ], F32, isOutput=False)
    b_ext = nc.declare_dram_parameter("b", [B_LOC, SEQ], F32, isOutput=False)
    o_ext = nc.declare_dram_parameter("out", [B_LOC, SEQ], F32, isOutput=True)

    with tile.TileContext(nc) as tc, ExitStack() as ctx:
        cpool = ctx.enter_context(tc.tile_pool(name="const", bufs=1))
        ten = cpool.tile([P, FS], F32)
        nc.vector.memset(ten[:], 10.0)
        # persistent ping-pong s tiles; separator cols written once
        s_pp = [cpool.tile([P, FS], F32, tag=f"s{i}", name=f"s_pp{i}")
                for i in range(N_SPP)]
        for s_t in s_pp:
            nc.vector.memset(s_t[:, 0:FS:GS], 0.0)

        io = ctx.enter_context(tc.tile_pool(name="io", bufs=IO_BUFS))
        wk = ctx.enter_context(tc.tile_pool(name="wk", bufs=WK_BUFS))

        base = 0
        for t, Gt in enumerate(G_LIST):
            FDt = Gt * SEQ
            FSt = Gt * GS + 1
            a_vt = a_ext[:][base:base + P * Gt].rearrange(
                "(p g) e -> p (g e)", p=P)
            b_vt = b_ext[:][base:base + P * Gt].rearrange(
                "(p g) e -> p (g e)", p=P)
            o_vt = o_ext[:][base:base + P * Gt].rearrange(
                "(p g) e -> p (g e)", p=P)
            base += P * Gt

            a_t = io.tile([P, FDt], F32, tag="a", name=f"a_{t}",
                          padded_shape=[P, FD])
            b_t = io.tile([P, FDt], F32, tag="b", name=f"b_{t}",
                          padded_shape=[P, FD])
            nc.sync.dma_start(out=a_t[:], in_=a_vt)
            nc.sync.dma_start(out=b_t[:], in_=b_vt)

            # s = a + b written into the LSB-first separator layout
            # (the digit reversal is folded into the output AP)
            s_full = s_pp[t % N_SPP]
            s_dj = s_full[:, 1:].rearrange(
                "p (g q) -> p g q", q=GS)[:, 0:Gt, 0:SEQ][:, :, ::-1]
            a3 = a_t[:].rearrange("p (g e) -> p g e", e=SEQ)
            b3 = b_t[:].rearrange("p (g e) -> p g e", e=SEQ)
            nc.vector.tensor_tensor(out=s_dj, in0=a3, in1=b3, op=ALU.add)

            # v_t = [10 <= v_{t-1}] + s_t : the whole carry chain
            # (scan state is fp32 internally; bf16 output exact for v<=19)
            w_t = wk.tile([P, FSt], BF16, tag="w", name=f"w_{t}",
                          padded_shape=[P, FS])
            nc.vector.tensor_tensor_scan(
                out=w_t[:], data0=ten[:, 0:FSt], data1=s_full[:, 0:FSt],
                initial=0.0, op0=ALU.is_le, op1=ALU.add)

            # m = -10*[v >= 10] (4x mode), then digit = m + v in place
            # (2x mode), all bf16, LSB-first
            g_t = wk.tile([P, FDt], BF16, tag="g", name=f"g_{t}",
                          padded_shape=[P, FD])
            w_data = w_t[:, 1:].rearrange("p (g q) -> p g q",
                                          q=GS)[:, :, 0:SEQ]
            g3 = g_t[:].rearrange("p (g e) -> p g e", e=SEQ)
            nc.vector.tensor_scalar(out=g3, in0=w_data, scalar1=10.0,
                                    scalar2=-10.0, op0=ALU.is_ge,
                                    op1=ALU.mult)
            nc.vector.tensor_tensor(out=g3, in0=g3, in1=w_data, op=ALU.add)

            # ACT upcasts to the f32 output tile, reversing back to
            # MSB-first digit order via its input access pattern
            d_t = wk.tile([P, FDt], F32, tag="d", name=f"d_{t}",
                          padded_shape=[P, FD])
            d3 = d_t[:].rearrange("p (g e) -> p g e", e=SEQ)
            nc.scalar.activation(d3, g3[:, :, ::-1], ACTF.Copy)

            nc.scalar.dma_start(out=o_vt, in_=d_t[:])

    nc.finalize()
    return nc


def kernel(a, b, weight_ih=None, weight_hh=None, bias_ih=None, bias_hh=None):
    """Full-batch digit adder. The RNN weights are the fixed carry-add
    weights baked into the module; the kernel implements that function
    directly, so they are accepted and unused."""
    from concourse.bass_utils import run_bass_kernel_spmd

    a = np.ascontiguousarray(np.asarray(a, dtype=np.float32))
    b = np.ascontiguousarray(np.asarray(b, dtype=np.float32))
    assert a.shape == (BATCH, SEQ) and b.shape == (BATCH, SEQ)

    if "nc" not in _nc_cache:
        _nc_cache["nc"] = _build_adder()
    nc = _nc_cache["nc"]

    in_maps = [
        {"a": a[i * B_LOC:(i + 1) * B_LOC],
         "b": b[i * B_LOC:(i + 1) * B_LOC]}
        for i in range(N_CORES)
    ]
    res = run_bass_kernel_spmd(nc, in_maps, core_ids=list(range(N_CORES)))
    return np.concatenate(
        [res.results[i]["out"] for i in range(N_CORES)], axis=0)


if __name__ == "__main__":
    rng = np.random.default_rng(0)
    a = rng.integers(0, 10, (BATCH, SEQ)).astype(np.float32)
    b = rng.integers(0, 10, (BATCH, SEQ)).astype(np.float32)
    out = kernel(a, b)
    # host reference
    c = np.zeros(BATCH, np.float32)
    exp = np.zeros_like(a)
    for e in range(SEQ - 1, -1, -1):
        s = a[:, e] + b[:, e] + c
        c = (s >= 10).astype(np.float32)
        exp[:, e] = s - 10 * c
    print("max abs err:", np.abs(out - exp).max())
